# revision 8
# baseline (speedup 1.0000x reference)
"""Trainium2 Bass kernel for nn_BinaryMNModel (binary Markov-network clique scoring).

Math: for each batch row b,
    ll[b] = sum_c sum_j f[c,j] * prod_s ( bc[j,s] ? x[b,vars[c,s]] : 1-x[b,vars[c,s]] )

We re-express each clique's factor table in the multilinear monomial basis
(a 8x8 +-1 transform of the 8 factor entries):
    score[c,b] = g0[c] + g1[c]*a0 + g2[c]*a1 + g3[c]*a2
               + g4[c]*a0*a1 + g5[c]*a0*a2 + g6[c]*a1*a2 + g7[c]*a0*a1*a2
with a_s = x[b, vars[c,s]].  Summing over cliques:
  - the constant term becomes one host-side scalar,
  - the linear terms fold into a V-length weight vector w (host scatter-add),
    so sum_c(linear) = x @ w  (done on-device, V-sharded across cores),
  - only the 4 quadratic/cubic monomials need the gathered values.

Sharding: cliques are sharded across the 8 cores (2500 each); the x@w matvec
is V-sharded.  Each core returns a partial [256] vector; host sums them.

Trace-driven structure (v4):
  - The gather phase is DESCRIPTOR-count bound (~2.6-3 ns per gathered row,
    dtype-independent), so the fp16 switch buys nothing there - but fp16
    halves DVE product time (2x_1p mode), runs PE matmuls at 1 cyc/row
    instead of 4, and halves SBUF pressure.  End-to-end rel err ~8e-4
    (harness gate 2e-2).
  - One dma_gather per 2-chunk group covering all 3 clique slots (10 calls
    x 768 idxs).  The SWDGE gather ucode breaks somewhere above 768
    idxs/call (1152 hangs the device), so 768 is the merge ceiling.
  - The Q7 gather-library load has ~9us latency and can only usefully
    start after the NEFF entry barrier (~6.5us); hoisting it before the
    barrier backfires (the barrier then waits on the load).  It is issued
    as the first main-block Pool instruction.
  - Products are computed per PAIR of gather groups with one 4D-AP DVE op
    per monomial (1024 cols, halves DVE instruction overhead), with enough
    product buffers that DVE never stalls on matmul consumption (the v3
    trace showed a DVE<->PE ping-pong tail from buffer WAR with bufs=4).
  - PE column-group chains 0/32/64 balanced (m012 alternates 32/64 by
    chunk parity) so the three chains end together.
"""

import os

import numpy as np

# ---------------------------------------------------------------- constants
B = 256
V = 5000
C = 20000
S = 3
NCOMB = 8
N_CORES = 8

C_SHARD = C // N_CORES          # 2500 cliques per core
CHUNKS = 20                     # 2560 = 20 * 128
C_PAD = CHUNKS * 128            # padded cliques per core

GC = 2                          # chunks per gather group (768 idxs/call)
NG = CHUNKS // GC               # 10 gather calls
PAIR_G = 2                      # gather groups per DVE product op
NPAIR = NG // PAIR_G            # 5 product steps
IDX_COLS = 3 * C_PAD // 16      # 480 columns in the wrapped idx layout

V_SHARD = V // N_CORES          # 625
V_CHUNKS = 5                    # padded to 640 = 5 * 128

# aux layout: [coef 4*CHUNKS | xv V_CHUNKS*B | wv V_CHUNKS]
COEF_OFF = 0
XV_OFF = 4 * CHUNKS
WV_OFF = XV_OFF + V_CHUNKS * B
AUX_COLS = WV_OFF + V_CHUNKS

_PROGRAM = None  # compiled program cache: (nc, out_name)

N_QUEUES = int(os.environ.get("K_NQ", "4"))
K_DT = os.environ.get("K_DT", "f16")


def _build_program():
    import concourse.mybir as mybir
    from concourse import bacc, library_config, tile

    f32 = mybir.dt.float32
    f16 = mybir.dt.float16 if K_DT == "f16" else mybir.dt.float32
    i16 = mybir.dt.int16
    MULT = mybir.AluOpType.mult

    nc = bacc.Bacc(
        "TRN2",
        target_bir_lowering=False,
        debug=False,
        enable_asserts=False,
        num_devices=N_CORES,
        num_swdge_queues=max(N_QUEUES, 1),
    )

    xt_d = nc.dram_tensor("xt", [V, B], f16, kind="ExternalInput")
    idx_d = nc.dram_tensor("idx", [128, IDX_COLS], i16, kind="ExternalInput")
    aux_d = nc.dram_tensor("aux", [128, AUX_COLS], f16, kind="ExternalInput")
    out_d = nc.dram_tensor("out", [1, B], f32, kind="ExternalOutput")

    with tile.TileContext(nc) as tc:
        with (
            tc.tile_pool(name="persist", bufs=1) as pp,
            tc.tile_pool(name="prod", bufs=NPAIR) as prodp,
            tc.tile_pool(name="ps", bufs=1, space="PSUM") as psp,
        ):
            # Q7 gather-library load: ~9us latency, issued first thing after
            # the entry barrier so it overlaps the idx/aux DMAs and the x@w
            # matvec.
            nc.gpsimd.load_library(library_config.mlp)

            idx_t = pp.tile([128, IDX_COLS], i16, tag="idx")
            aux_t = pp.tile([128, AUX_COLS], f16, tag="aux")
            # gather group g owns cols [:, g, :, :]: 6 chunk-cols, slot-major
            # (col j = s*GC + c within the group)
            a_all = pp.tile([128, NG, S * GC, B], f16, tag="a_all")
            out_sb = pp.tile([1, B], f32, tag="out_sb")
            tmp_s = pp.tile([1, B], f32, tag="tmp_s")
            tmp_t = pp.tile([1, B], f32, tag="tmp_t")
            psum_t = psp.tile([128, B], f32, tag="psum")

            nc.sync.dma_start(idx_t[:], idx_d[:])
            nc.sync.dma_start(aux_t[:], aux_d[:])

            # PE accumulation: weighted clique reductions run concurrently in
            # the PE's 128x32 column-groups (col-group 3 is a known HW bug, so
            # only groups 0/32/64 are used).  m01 -> row 0 (after the x@w
            # matvec), m02 -> row 32, m12 -> row 64; m012 alternates rows
            # 32/64 by chunk parity so the chains end together.
            row_started = set()

            def mm(mono, chunk, moving_ap, last=False):
                coef_col = mono * CHUNKS + chunk
                lhs = aux_t[:, COEF_OFF + coef_col : COEF_OFF + coef_col + 1]
                if mono < 3:
                    row = 32 * mono
                else:
                    row = 32 if chunk % 2 == 0 else 64
                nc.tensor.matmul(
                    psum_t[row : row + 1, :],
                    lhs,
                    moving_ap,
                    start=(row not in row_started),
                    stop=last,
                    tile_position=(0, row),
                )
                row_started.add(row)

            # all gathers first: they are gpsimd's only work and pace the
            # rest.  queue assignment must be pure round-robin: Tile's 8
            # DMASW sem lanes are assigned round-robin per call and each
            # lane is locked to one SWDGE queue.
            for g in range(NG):
                n_idx = 3 * GC * 128
                nc.gpsimd.dma_gather(
                    a_all[:, g, :, :],
                    xt_d[:],
                    idx_t[:, 24 * GC * g : 24 * GC * (g + 1)],
                    n_idx,
                    n_idx,
                    B,
                    queue_num=g % N_QUEUES,
                )

            # linear terms: x @ w on the PE row-0 chain (data ready early,
            # runs while the gather library is still loading)
            for j in range(V_CHUNKS):
                nc.tensor.matmul(
                    psum_t[0:1, :],
                    aux_t[:, WV_OFF + j : WV_OFF + j + 1],
                    aux_t[:, XV_OFF + j * B : XV_OFF + (j + 1) * B],
                    start=(j == 0),
                    stop=False,
                    tile_position=(0, 0),
                )
            row_started.add(0)

            for p in range(NPAIR):
                g0 = p * PAIR_G
                gsl = slice(g0, g0 + PAIR_G)
                a0g = a_all[:, gsl, 0 * GC : 1 * GC, :]
                a1g = a_all[:, gsl, 1 * GC : 2 * GC, :]
                a2g = a_all[:, gsl, 2 * GC : 3 * GC, :]
                p01 = prodp.tile([128, PAIR_G, GC, B], f16, tag="p01", name="p01")
                p02 = prodp.tile([128, PAIR_G, GC, B], f16, tag="p02", name="p02")
                p12 = prodp.tile([128, PAIR_G, GC, B], f16, tag="p12", name="p12")
                p012 = prodp.tile([128, PAIR_G, GC, B], f16, tag="p012", name="p012")
                nc.vector.tensor_tensor(p01[:], a0g, a1g, MULT)
                nc.vector.tensor_tensor(p02[:], a0g, a2g, MULT)
                nc.vector.tensor_tensor(p12[:], a1g, a2g, MULT)
                nc.vector.tensor_tensor(p012[:], p01[:], a2g, MULT)
                lastp = p == NPAIR - 1
                for i in range(PAIR_G):
                    for c in range(GC):
                        ci = (g0 + i) * GC + c
                        lc = lastp and i == PAIR_G - 1 and c == GC - 1
                        mm(0, ci, p01[:, i, c, :], last=lc)   # row 0 ends m01
                        mm(1, ci, p02[:, i, c, :], last=lc)   # row 32 ends m02
                        mm(2, ci, p12[:, i, c, :])
                        mm(3, ci, p012[:, i, c, :], last=lc)  # row 64 ends m012
                        # chunk 19 is odd -> its m012 goes to row 64, and it is
                        # issued after m12(c19), so the stop flags are correct.

            # combine the 3 chain rows (0, 32, 64) into the output
            # (DVE may read at most one PSUM operand per instruction)
            nc.vector.tensor_copy(tmp_s[:], psum_t[0:1, :])
            nc.vector.tensor_add(tmp_t[:], tmp_s[:], psum_t[32:33, :])
            nc.vector.tensor_add(out_sb[:], tmp_t[:], psum_t[64:65, :])
            nc.sync.dma_start(out_d[:], out_sb[:])

    nc.compile()
    return nc, out_d.name


def get_program():
    global _PROGRAM
    if _PROGRAM is None:
        _PROGRAM = _build_program()
    return _PROGRAM


# ---------------------------------------------------------------- host prep
def _monomial_transform(all_factors: np.ndarray) -> np.ndarray:
    """g[c,t] such that score[c,b] = sum_t g[c,t] * prod_{s: bit (S-1-s) of t} a_s."""
    M = np.zeros((NCOMB, NCOMB), dtype=np.float64)
    for t in range(NCOMB):
        for j in range(NCOMB):
            if j & ~t:
                continue
            M[t, j] = (-1.0) ** bin(t & ~j).count("1")
    return all_factors.astype(np.float64) @ M.T


def _wrap_idx(idx: np.ndarray) -> np.ndarray:
    """[N] int -> [128, N/16] int16 dma_gather layout (idx i at partition
    i%16, col i//16, replicated across the 8 q7 cores)."""
    w = idx.reshape(-1, 16).T.astype(np.int16)  # [16, N/16]
    return np.tile(w, (8, 1))


def _chunk_layout(v: np.ndarray) -> np.ndarray:
    """[C_PAD] -> [128, CHUNKS]: element i at partition i%128, col i//128."""
    return np.ascontiguousarray(v.reshape(CHUNKS, 128).T)


def prepare_inputs(x, all_vars, all_factors):
    np_dt = np.float16 if K_DT == "f16" else np.float32
    x = np.asarray(x, dtype=np.float32)
    all_vars = np.asarray(all_vars)
    all_factors = np.asarray(all_factors, dtype=np.float32)

    xt = np.ascontiguousarray(x.T.astype(np_dt))  # [V, B]

    g = _monomial_transform(all_factors)  # [C, 8] f64
    bit = [1 << (S - 1 - s) for s in range(S)]
    t01, t02, t12 = bit[0] | bit[1], bit[0] | bit[2], bit[1] | bit[2]
    t012 = bit[0] | bit[1] | bit[2]

    const0 = float(g[:, 0].sum())
    w = np.zeros(V, dtype=np.float64)
    for s in range(S):
        np.add.at(w, all_vars[:, s], g[:, bit[s]])
    w = w.astype(np_dt)
    g_lo = g.astype(np_dt)

    in_maps = []
    for k in range(N_CORES):
        sl = slice(k * C_SHARD, (k + 1) * C_SHARD)
        pad = C_PAD - C_SHARD

        # vars_chunk[s][p, c] = all_vars[128*c + p, s] (padded with 0)
        vars_chunk = []
        for s in range(S):
            vv = np.concatenate([all_vars[sl, s], np.zeros(pad, np.int64)])
            vars_chunk.append(vv.reshape(CHUNKS, 128).T)  # [128, CHUNKS]

        # merged per-group idx layout: for group g (chunks c0..c1), concat
        # over slots s of the chunk columns (p fastest), wrapped into 16
        # partitions
        idx_parts = []
        for gi in range(NG):
            c0, c1 = gi * GC, (gi + 1) * GC
            seq = np.concatenate(
                [vars_chunk[s][:, c].ravel() for s in range(S) for c in range(c0, c1)]
            )
            idx_parts.append(_wrap_idx(seq))
        idx_arr = np.ascontiguousarray(np.concatenate(idx_parts, axis=1))
        assert idx_arr.shape == (128, IDX_COLS)

        coef_cols = []
        for t in (t01, t02, t12, t012):
            gg = np.concatenate([g_lo[sl, t].astype(np_dt), np.zeros(pad, np_dt)])
            coef_cols.append(_chunk_layout(gg))
        coef_arr = np.concatenate(coef_cols, axis=1)  # [128, 4*CHUNKS]

        vs = slice(k * V_SHARD, (k + 1) * V_SHARD)
        vpad = V_CHUNKS * 128 - V_SHARD
        xv = np.concatenate([xt[vs], np.zeros((vpad, B), np_dt)])
        xv = xv.reshape(V_CHUNKS, 128, B).transpose(1, 0, 2).reshape(128, V_CHUNKS * B)
        wv = np.concatenate([w[vs], np.zeros(vpad, np_dt)])
        wv = np.ascontiguousarray(wv.reshape(V_CHUNKS, 128).T)

        aux = np.ascontiguousarray(
            np.concatenate([coef_arr, xv, wv], axis=1, dtype=np_dt)
        )
        assert aux.shape == (128, AUX_COLS)
        in_maps.append({"xt": xt, "idx": idx_arr, "aux": aux})

    return in_maps, const0


# ---------------------------------------------------------------- entry
def run(inputs: dict, trace: bool = False):
    from concourse import bass_utils

    in_maps, const0 = prepare_inputs(
        inputs["x"], inputs["all_vars"], inputs["all_factors"]
    )
    nc, out_name = get_program()
    res = bass_utils.run_bass_kernel_spmd(
        nc, in_maps, core_ids=list(range(N_CORES)), trace=trace
    )
    partials = np.stack([np.asarray(r[out_name]).reshape(B) for r in res.results])
    ll = partials.astype(np.float64).sum(axis=0) + const0
    return ll.astype(np.float32), res


def kernel(x, binary_combinations, all_vars, all_factors):
    out, _ = run(
        {"x": x, "all_vars": all_vars, "all_factors": all_factors}
    )
    return out


# revision 10
# speedup vs baseline: 1.1981x; 1.1981x over previous
"""Trainium2 Bass kernel for nn_BinaryMNModel (binary Markov-network clique scoring).

Math: for each batch row b,
    ll[b] = sum_c sum_j f[c,j] * prod_s ( bc[j,s] ? x[b,vars[c,s]] : 1-x[b,vars[c,s]] )

We re-express each clique's factor table in the multilinear monomial basis
(a 8x8 +-1 transform of the 8 factor entries):
    score[c,b] = g0[c] + g1[c]*a0 + g2[c]*a1 + g3[c]*a2
               + g4[c]*a0*a1 + g5[c]*a0*a2 + g6[c]*a1*a2 + g7[c]*a0*a1*a2
with a_s = x[b, vars[c,s]].  Summing over cliques:
  - the constant term becomes one host-side scalar,
  - the linear terms fold into a V-length weight vector w (host scatter-add),
    so sum_c(linear) = x @ w  (done on-device, V-sharded across cores),
  - only the 4 quadratic/cubic monomials need the gathered values.

Sharding: cliques are sharded across the 8 cores (2500 each); the x@w matvec
is V-sharded.  Each core returns a partial [256] vector; host sums them.

Trace-driven structure (v4):
  - The gather phase is DESCRIPTOR-count bound (~2.6-3 ns per gathered row,
    dtype-independent), so the fp16 switch buys nothing there - but fp16
    halves DVE product time (2x_1p mode), runs PE matmuls at 1 cyc/row
    instead of 4, and halves SBUF pressure.  End-to-end rel err ~8e-4
    (harness gate 2e-2).
  - One dma_gather per 2-chunk group covering all 3 clique slots (10 calls
    x 768 idxs).  The SWDGE gather ucode breaks somewhere above 768
    idxs/call (1152 hangs the device), so 768 is the merge ceiling.
  - The Q7 gather-library load has ~9us latency and can only usefully
    start after the NEFF entry barrier (~6.5us); hoisting it before the
    barrier backfires (the barrier then waits on the load).  It is issued
    as the first main-block Pool instruction.
  - Products are computed per PAIR of gather groups with one 4D-AP DVE op
    per monomial (1024 cols, halves DVE instruction overhead), with enough
    product buffers that DVE never stalls on matmul consumption (the v3
    trace showed a DVE<->PE ping-pong tail from buffer WAR with bufs=4).
  - PE column-group chains 0/32/64 balanced (m012 alternates 32/64 by
    chunk parity) so the three chains end together.
"""

import os

import numpy as np

# ---------------------------------------------------------------- constants
B = 256
V = 5000
C = 20000
S = 3
NCOMB = 8
N_CORES = 8

C_SHARD = C // N_CORES          # 2500 cliques per core
CHUNKS = 20                     # 2560 = 20 * 128
C_PAD = CHUNKS * 128            # padded cliques per core

# gather group sizes in chunks: one dma_gather per group covering all 3
# slots (<=768 idxs per call; the SWDGE ucode breaks above that).  12 calls
# keep all 4 SWDGE queues busy (queue = call % 4) and the small first groups
# get product data flowing as soon as the Q7 library is up.
_GC_ENV = os.environ.get("K_GC", "")
if _GC_ENV:
    GROUP_CHUNKS = [int(v) for v in _GC_ENV.split(",")]
else:
    GROUP_CHUNKS = [1, 1, 1, 1] + [2] * 8
assert sum(GROUP_CHUNKS) == CHUNKS
K_BUFS = int(os.environ.get("K_BUFS", "8"))
IDX_COLS = 3 * C_PAD // 16      # 480 columns in the wrapped idx layout

V_SHARD = V // N_CORES          # 625
V_CHUNKS = 5                    # padded to 640 = 5 * 128

# aux layout: [coef 4*CHUNKS | xv V_CHUNKS*B | wv V_CHUNKS]
COEF_OFF = 0
XV_OFF = 4 * CHUNKS
WV_OFF = XV_OFF + V_CHUNKS * B
AUX_COLS = WV_OFF + V_CHUNKS

_PROGRAM = None  # compiled program cache: (nc, out_name)

N_QUEUES = int(os.environ.get("K_NQ", "4"))
K_DT = os.environ.get("K_DT", "f16")


def _build_program():
    import concourse.mybir as mybir
    from concourse import bacc, library_config, tile

    f32 = mybir.dt.float32
    f16 = mybir.dt.float16 if K_DT == "f16" else mybir.dt.float32
    i16 = mybir.dt.int16
    MULT = mybir.AluOpType.mult

    nc = bacc.Bacc(
        "TRN2",
        target_bir_lowering=False,
        debug=False,
        enable_asserts=False,
        num_devices=N_CORES,
        num_swdge_queues=max(N_QUEUES, 1),
    )

    xt_d = nc.dram_tensor("xt", [V, B], f16, kind="ExternalInput")
    idx_d = nc.dram_tensor("idx", [128, IDX_COLS], i16, kind="ExternalInput")
    aux_d = nc.dram_tensor("aux", [128, AUX_COLS], f16, kind="ExternalInput")
    out_d = nc.dram_tensor("out", [1, B], f32, kind="ExternalOutput")

    with tile.TileContext(nc) as tc:
        with (
            tc.tile_pool(name="persist", bufs=1) as pp,
            tc.tile_pool(name="prod", bufs=K_BUFS) as prodp,
            tc.tile_pool(name="ps", bufs=1, space="PSUM") as psp,
        ):
            # Q7 gather-library load: ~9us latency, issued first thing after
            # the entry barrier so it overlaps the idx/aux DMAs and the x@w
            # matvec.
            nc.gpsimd.load_library(library_config.mlp)

            idx_t = pp.tile([128, IDX_COLS], i16, tag="idx")
            aux_t = pp.tile([128, AUX_COLS], f16, tag="aux")
            # gather group g (chunks c0..c1) owns cols [3*c0, 3*c1),
            # slot-major within the group (col = 3*c0 + s*gc + (c-c0))
            a_all = pp.tile([128, 3 * CHUNKS, B], f16, tag="a_all")
            out_sb = pp.tile([1, B], f32, tag="out_sb")
            tmp_s = pp.tile([1, B], f32, tag="tmp_s")
            tmp_t = pp.tile([1, B], f32, tag="tmp_t")
            psum_t = psp.tile([128, B], f32, tag="psum")

            nc.sync.dma_start(idx_t[:], idx_d[:])
            nc.sync.dma_start(aux_t[:], aux_d[:])

            # PE accumulation: weighted clique reductions run concurrently in
            # the PE's 128x32 column-groups (col-group 3 is a known HW bug, so
            # only groups 0/32/64 are used).  m01 -> row 0 (after the x@w
            # matvec), m02 -> row 32, m12 -> row 64; m012 alternates rows
            # 32/64 by chunk parity so the chains end together.
            row_started = set()

            def mm(mono, chunk, moving_ap, last=False):
                coef_col = mono * CHUNKS + chunk
                lhs = aux_t[:, COEF_OFF + coef_col : COEF_OFF + coef_col + 1]
                if mono < 3:
                    row = 32 * mono
                else:
                    row = 32 if chunk % 2 == 0 else 64
                nc.tensor.matmul(
                    psum_t[row : row + 1, :],
                    lhs,
                    moving_ap,
                    start=(row not in row_started),
                    stop=last,
                    tile_position=(0, row),
                )
                row_started.add(row)

            # all gathers first: they are gpsimd's only work and pace the
            # rest.  queue assignment must be pure round-robin: Tile's 8
            # DMASW sem lanes are assigned round-robin per call and each
            # lane is locked to one SWDGE queue.
            c_off = [0]
            for gc in GROUP_CHUNKS:
                c_off.append(c_off[-1] + gc)
            for g, gc in enumerate(GROUP_CHUNKS):
                c0, c1 = c_off[g], c_off[g + 1]
                n_idx = 3 * gc * 128
                nc.gpsimd.dma_gather(
                    a_all[:, 3 * c0 : 3 * c1, :],
                    xt_d[:],
                    idx_t[:, 24 * c0 : 24 * c1],
                    n_idx,
                    n_idx,
                    B,
                    queue_num=g % N_QUEUES,
                )

            # linear terms: x @ w on the PE row-0 chain (data ready early,
            # runs while the gather library is still loading)
            for j in range(V_CHUNKS):
                nc.tensor.matmul(
                    psum_t[0:1, :],
                    aux_t[:, WV_OFF + j : WV_OFF + j + 1],
                    aux_t[:, XV_OFF + j * B : XV_OFF + (j + 1) * B],
                    start=(j == 0),
                    stop=False,
                    tile_position=(0, 0),
                )
            row_started.add(0)

            for g, gc in enumerate(GROUP_CHUNKS):
                c0, c1 = c_off[g], c_off[g + 1]
                off = 3 * c0
                a0g = a_all[:, off : off + gc, :]
                a1g = a_all[:, off + gc : off + 2 * gc, :]
                a2g = a_all[:, off + 2 * gc : off + 3 * gc, :]
                p01 = prodp.tile([128, gc, B], f16, tag="p01", name="p01")
                p02 = prodp.tile([128, gc, B], f16, tag="p02", name="p02")
                p12 = prodp.tile([128, gc, B], f16, tag="p12", name="p12")
                p012 = prodp.tile([128, gc, B], f16, tag="p012", name="p012")
                nc.vector.tensor_tensor(p01[:], a0g, a1g, MULT)
                nc.vector.tensor_tensor(p02[:], a0g, a2g, MULT)
                nc.vector.tensor_tensor(p12[:], a1g, a2g, MULT)
                nc.vector.tensor_tensor(p012[:], p01[:], a2g, MULT)
                lastg = g == len(GROUP_CHUNKS) - 1
                for c in range(gc):
                    ci = c_off[g] + c
                    lc = lastg and c == gc - 1
                    mm(0, ci, p01[:, c, :], last=lc)   # row 0 ends m01
                    mm(1, ci, p02[:, c, :], last=lc)   # row 32 ends m02
                    mm(2, ci, p12[:, c, :])
                    mm(3, ci, p012[:, c, :], last=lc)  # row 64 ends m012
                    # chunk 19 is odd -> its m012 goes to row 64, and it is
                    # issued after m12(c19), so the stop flags are correct.

            # combine the 3 chain rows (0, 32, 64) into the output
            # (DVE may read at most one PSUM operand per instruction)
            nc.vector.tensor_copy(tmp_s[:], psum_t[0:1, :])
            nc.vector.tensor_add(tmp_t[:], tmp_s[:], psum_t[32:33, :])
            nc.vector.tensor_add(out_sb[:], tmp_t[:], psum_t[64:65, :])
            nc.sync.dma_start(out_d[:], out_sb[:])

    nc.compile()
    return nc, out_d.name


def get_program():
    global _PROGRAM
    if _PROGRAM is None:
        _PROGRAM = _build_program()
    return _PROGRAM


# ---------------------------------------------------------------- host prep
def _monomial_transform(all_factors: np.ndarray) -> np.ndarray:
    """g[c,t] such that score[c,b] = sum_t g[c,t] * prod_{s: bit (S-1-s) of t} a_s."""
    M = np.zeros((NCOMB, NCOMB), dtype=np.float64)
    for t in range(NCOMB):
        for j in range(NCOMB):
            if j & ~t:
                continue
            M[t, j] = (-1.0) ** bin(t & ~j).count("1")
    return all_factors.astype(np.float64) @ M.T


def _wrap_idx(idx: np.ndarray) -> np.ndarray:
    """[N] int -> [128, N/16] int16 dma_gather layout (idx i at partition
    i%16, col i//16, replicated across the 8 q7 cores)."""
    w = idx.reshape(-1, 16).T.astype(np.int16)  # [16, N/16]
    return np.tile(w, (8, 1))


def _chunk_layout(v: np.ndarray) -> np.ndarray:
    """[C_PAD] -> [128, CHUNKS]: element i at partition i%128, col i//128."""
    return np.ascontiguousarray(v.reshape(CHUNKS, 128).T)


def prepare_inputs(x, all_vars, all_factors):
    np_dt = np.float16 if K_DT == "f16" else np.float32
    x = np.asarray(x, dtype=np.float32)
    all_vars = np.asarray(all_vars)
    all_factors = np.asarray(all_factors, dtype=np.float32)

    xt = np.ascontiguousarray(x.T.astype(np_dt))  # [V, B]

    g = _monomial_transform(all_factors)  # [C, 8] f64
    bit = [1 << (S - 1 - s) for s in range(S)]
    t01, t02, t12 = bit[0] | bit[1], bit[0] | bit[2], bit[1] | bit[2]
    t012 = bit[0] | bit[1] | bit[2]

    const0 = float(g[:, 0].sum())
    w = np.zeros(V, dtype=np.float64)
    for s in range(S):
        np.add.at(w, all_vars[:, s], g[:, bit[s]])
    w = w.astype(np_dt)
    g_lo = g.astype(np_dt)

    in_maps = []
    for k in range(N_CORES):
        sl = slice(k * C_SHARD, (k + 1) * C_SHARD)
        pad = C_PAD - C_SHARD

        # vars_chunk[s][p, c] = all_vars[128*c + p, s] (padded with 0)
        vars_chunk = []
        for s in range(S):
            vv = np.concatenate([all_vars[sl, s], np.zeros(pad, np.int64)])
            vars_chunk.append(vv.reshape(CHUNKS, 128).T)  # [128, CHUNKS]

        # merged per-group idx layout: for group g (chunks c0..c1), concat
        # over slots s of the chunk columns (p fastest), wrapped into 16
        # partitions
        idx_parts = []
        co = 0
        for gc in GROUP_CHUNKS:
            c0, c1 = co, co + gc
            co += gc
            seq = np.concatenate(
                [vars_chunk[s][:, c].ravel() for s in range(S) for c in range(c0, c1)]
            )
            idx_parts.append(_wrap_idx(seq))
        idx_arr = np.ascontiguousarray(np.concatenate(idx_parts, axis=1))
        assert idx_arr.shape == (128, IDX_COLS)

        coef_cols = []
        for t in (t01, t02, t12, t012):
            gg = np.concatenate([g_lo[sl, t].astype(np_dt), np.zeros(pad, np_dt)])
            coef_cols.append(_chunk_layout(gg))
        coef_arr = np.concatenate(coef_cols, axis=1)  # [128, 4*CHUNKS]

        vs = slice(k * V_SHARD, (k + 1) * V_SHARD)
        vpad = V_CHUNKS * 128 - V_SHARD
        xv = np.concatenate([xt[vs], np.zeros((vpad, B), np_dt)])
        xv = xv.reshape(V_CHUNKS, 128, B).transpose(1, 0, 2).reshape(128, V_CHUNKS * B)
        wv = np.concatenate([w[vs], np.zeros(vpad, np_dt)])
        wv = np.ascontiguousarray(wv.reshape(V_CHUNKS, 128).T)

        aux = np.ascontiguousarray(
            np.concatenate([coef_arr, xv, wv], axis=1, dtype=np_dt)
        )
        assert aux.shape == (128, AUX_COLS)
        in_maps.append({"xt": xt, "idx": idx_arr, "aux": aux})

    return in_maps, const0


# ---------------------------------------------------------------- entry
def run(inputs: dict, trace: bool = False):
    from concourse import bass_utils

    in_maps, const0 = prepare_inputs(
        inputs["x"], inputs["all_vars"], inputs["all_factors"]
    )
    nc, out_name = get_program()
    res = bass_utils.run_bass_kernel_spmd(
        nc, in_maps, core_ids=list(range(N_CORES)), trace=trace
    )
    partials = np.stack([np.asarray(r[out_name]).reshape(B) for r in res.results])
    ll = partials.astype(np.float64).sum(axis=0) + const0
    return ll.astype(np.float32), res


def kernel(x, binary_combinations, all_vars, all_factors):
    out, _ = run(
        {"x": x, "all_vars": all_vars, "all_factors": all_factors}
    )
    return out
